# revision 13
# baseline (speedup 1.0000x reference)
"""Trainium2 Bass kernel for nn_LinearNNEncoder (fused Linear+GELU, masked per-batch
mean/std over ragged sequences), data-parallel over 8 NeuronCores.

Contract: kernel(**inputs) takes FULL inputs (x [64,2048,300] f32, W [300,300],
b [300]) and returns the FULL output [64, 600] f32 (concat(std, mean) per batch).

Design (v3):
  - Host drops all-padding 128-token tiles (ragged lengths -> ~0.65x work), zeroes
    the remaining pad rows, and packs tiles into batch-aligned GROUPS of 1..4 tiles.
    Every core receives the same sorted multiset of group sizes (dummy all-zero
    groups pad each size class to a multiple of 8), so one SPMD program serves all
    cores while per-core data differs.
  - Per group: one dma_start_transpose loads x^T [128, 3, 128*sz] bf16 straight
    from DRAM (HW xbar transpose; d on partitions).  PE runs 9 matmuls (W-slice
    stationary [128,128] bf16, x^T moving) accumulating pre-activations per
    128-wide output slice in PSUM.  ScalarE applies exact GELU with the bias as a
    per-partition vector and emits the per-slice token-sum via accum_out (fp32
    datapath).  VectorE squares y (scalar_tensor_tensor) and emits sum(y^2) via
    accum_out.  Sums land in per-(slice, group) slots; one DMA returns them.
  - Host epilogue: sums slots per batch, subtracts the analytic contribution of
    the zeroed pad rows (gelu(b) per token; bf16-rounded for the square sum), and
    computes mean/std (unbiased, n<=1 and NaN edge cases per the reference).
"""
import numpy as np
import ml_dtypes

B, T, D = 64, 2048, 300
NCORES = 8
P = 128
DP = 384          # 3 k-slices of 128 (cols 300..383 zero)
KT = 3
MAXSZ = 4         # max tiles per group

bf16 = ml_dtypes.bfloat16

_cache = {}


def _build_nc(group_sizes):
    """One SPMD program for the given per-core group-size list (sorted desc)."""
    from contextlib import ExitStack
    import concourse.tile as tile
    from concourse import mybir, bacc

    f32 = mybir.dt.float32
    bf = mybir.dt.bfloat16
    AF = mybir.ActivationFunctionType
    OP = mybir.AluOpType

    G = len(group_sizes)
    total_tok = 128 * sum(group_sizes)
    PF = 3  # DMA prefetch depth (groups)

    nc = bacc.Bacc("TRN2", target_bir_lowering=False, debug=False)
    x_dram = nc.dram_tensor("x", [total_tok, DP], bf, kind="ExternalInput")
    wt_dram = nc.dram_tensor("wt", [P, KT * KT, P], bf, kind="ExternalInput")
    bn_dram = nc.dram_tensor("bn", [P, KT, G, 6], f32, kind="ExternalOutput")

    offs = np.concatenate([[0], np.cumsum(np.asarray(group_sizes) * 128)])

    with ExitStack() as ctx:
        tc = ctx.enter_context(tile.TileContext(nc))
        const = ctx.enter_context(tc.tile_pool(name="const", bufs=1))
        xtp = ctx.enter_context(tc.tile_pool(name="xtp", bufs=PF + 2))
        yp = ctx.enter_context(tc.tile_pool(name="yp", bufs=3))
        ps_y = ctx.enter_context(tc.tile_pool(name="ps_y", bufs=2, space="PSUM"))
        ps_w = ctx.enter_context(tc.tile_pool(name="ps_w", bufs=1, space="PSUM"))

        wt_sb = const.tile([P, KT * KT, P], bf)
        nc.sync.dma_start(wt_sb[:], wt_dram.ap())
        bnacc = const.tile([P, KT, G, 6], f32)

        # PE warmup: ramp HAM toward 8/8 while the first x DMAs are in flight.
        pwu = ps_w.tile([P, 2 * P], f32, name="pwu", tag="warm")
        for w in range(10):
            nc.tensor.matmul(pwu[:, (w % 2) * P:(w % 2) * P + P],
                             wt_sb[:, w % (KT * KT), :], wt_sb[:, 0, :])

        xts = {}

        def fetch(g):
            sz = group_sizes[g]
            xt = xtp.tile([P, KT, 128 * sz], bf, name=f"xt{g}", tag="xt")
            nc.sync.dma_start_transpose(
                xt[:], x_dram.ap()[offs[g]:offs[g] + 128 * sz, :])
            xts[g] = xt

        for g in range(min(PF, G)):
            fetch(g)

        for g in range(G):
            sz = group_sizes[g]
            n = 128 * sz
            if g + PF < G:
                fetch(g + PF)
            xt = xts.pop(g)
            # full 3-bank tile: each 512-entry slice is one PSUM bank, so the
            # matmul outputs stay bank-aligned even for tail groups (n < 512)
            py = ps_y.tile([P, KT, 512], f32, name=f"py{g}", tag="py")
            for i in range(KT):
                for j in range(KT):
                    nc.tensor.matmul(
                        py[:, i, 0:n], wt_sb[:, KT * i + j, :], xt[:, j, :],
                        start=(j == 0), stop=(j == KT - 1),
                    )
            y_sb = yp.tile([P, KT, n], bf, name=f"y{g}", tag="y")
            nc.scalar.activation(y_sb[:], py[:, :, 0:n], AF.Gelu)
            for i in range(KT):
                nc.vector.bn_stats(bnacc[:, i, g, :], y_sb[:, i, :])
        nc.sync.dma_start(bn_dram.ap()[:], bnacc[:])

    nc.compile()
    return nc


def _plan(valid):
    """Build the packing schedule from the validity mask [B, T].

    Returns (group_sizes, per-core schedules).  Each schedule entry is
    (batch, tile_indices) for one group; batch < 0 marks a dummy group."""
    TPB = T // 128
    vt = valid.reshape(B, TPB, 128)
    keep = vt.any(axis=2)  # [B, TPB] tiles with >=1 valid token

    groups = []  # (size, batch, tile_idx_list)
    for b in range(B):
        tiles = np.nonzero(keep[b])[0].tolist()
        for k in range(0, len(tiles), MAXSZ):
            chunk = tiles[k:k + MAXSZ]
            groups.append((len(chunk), b, chunk))

    # pad each size class to a multiple of NCORES with dummy groups
    from collections import Counter
    cnt = Counter(g[0] for g in groups)
    for s in list(cnt):
        for _ in range((-cnt[s]) % NCORES):
            groups.append((s, -1, []))
    groups.sort(key=lambda g: -g[0])

    # deal round-robin: core c takes groups c, c+8, ... -> identical size lists
    scheds = [[] for _ in range(NCORES)]
    for idx, g in enumerate(groups):
        scheds[idx % NCORES].append(g)
    group_sizes = tuple(g[0] for g in scheds[0])
    for c in range(1, NCORES):
        assert tuple(g[0] for g in scheds[c]) == group_sizes
    return group_sizes, scheds


def _pack_inputs(x, W, b, valid, group_sizes, scheds):
    """Build per-core input maps and the slot->batch bookkeeping."""
    total_tok = 128 * sum(group_sizes)
    vt = valid.reshape(B, T // 128, 128)

    wt = np.zeros((P, KT * KT, P), np.float32)
    Wp = np.zeros((DP, DP), np.float32)
    Wp[:D, :D] = np.asarray(W, np.float32)
    Wp[:D, DP - 1] = np.asarray(b, np.float32)  # bias via ones-column
    for i in range(KT):
        for j in range(KT):
            wt[:, KT * i + j, :] = Wp[128 * i:128 * i + 128,
                                      128 * j:128 * j + 128].T
    wtb = wt.astype(bf16)

    x32 = np.asarray(x, np.float32)
    in_maps = []
    slot_info = []  # per core: list of batch per group slot (-1 = dummy)
    for c in range(NCORES):
        xc = np.zeros((total_tok, DP), bf16)
        info = []
        row = 0
        for (sz, bidx, tiles) in scheds[c]:
            for t in tiles:
                seg = x32[bidx, t * 128:(t + 1) * 128, :]  # [128, 300]
                v = vt[bidx, t]  # [128] bool
                xc[row:row + 128, :D] = np.where(
                    v[:, None], seg, 0.0).astype(bf16)
                xc[row:row + 128, DP - 1] = v.astype(bf16)  # ones col
                row += 128
            row += 128 * (sz - len(tiles))  # dummy groups stay zero
            info.append(bidx)
        in_maps.append({"x": xc, "wt": wtb})
        slot_info.append(info)
    return in_maps, slot_info


def _host_epilogue(res, slot_info, n_valid, b):
    # pad rows are all-zero incl. the ones-column -> y = gelu(0) = 0 exactly,
    # so slot sums need no correction.
    S = np.zeros((B, DP), np.float64)
    Q = np.zeros((B, DP), np.float64)
    for c in range(NCORES):
        bn = np.asarray(res[c]["bn"], np.float64)  # [128, 3, G, 6]
        s_all = bn[..., 0] * bn[..., 1] + bn[..., 3] * bn[..., 4]
        q_all = (bn[..., 2] + bn[..., 0] * np.square(bn[..., 1])
                 + bn[..., 5] + bn[..., 3] * np.square(bn[..., 4]))
        for g, bidx in enumerate(slot_info[c]):
            if bidx < 0:
                continue
            S[bidx] += s_all[:, :, g].T.reshape(DP)
            Q[bidx] += q_all[:, :, g].T.reshape(DP)

    S = S[:, :D]
    Q = Q[:, :D]
    n = n_valid.astype(np.float64)[:, None]

    with np.errstate(divide="ignore", invalid="ignore"):
        mean = S / n
        var = (Q - S * S / n) / np.maximum(n - 1.0, 1.0)
        std = np.where(n > 1.0, np.sqrt(np.maximum(var, 0.0)), 0.0)
    out = np.concatenate([std, mean], axis=-1).astype(np.float32)
    return np.where(np.isnan(out), np.float32(0.0), out)


def _prep(x, W, b):
    x32 = np.asarray(x, np.float32)
    valid = ~np.all(x32 == -1.0, axis=2)  # [B, T]
    group_sizes, scheds = _plan(valid)
    in_maps, slot_info = _pack_inputs(x, W, b, valid, group_sizes, scheds)
    return group_sizes, in_maps, slot_info, valid.sum(1)


def _trace_in_maps(ins):
    group_sizes, in_maps, _, _ = _prep(**ins)
    return in_maps


def kernel(x, W, b):
    from concourse.bass_utils import run_bass_kernel_spmd

    group_sizes, in_maps, slot_info, n_valid = _prep(x, W, b)
    if group_sizes not in _cache:
        _cache[group_sizes] = _build_nc(list(group_sizes))
        _cache["nc"] = _cache[group_sizes]  # latest, for test.py tracing
    nc = _cache[group_sizes]
    _cache["nc"] = nc

    res = run_bass_kernel_spmd(nc, in_maps, core_ids=list(range(NCORES)))
    return _host_epilogue(res.results, slot_info, n_valid, b)


# revision 17
# speedup vs baseline: 1.0970x; 1.0970x over previous
"""Trainium2 Bass kernel for nn_LinearNNEncoder (fused Linear+GELU, masked per-batch
mean/std over ragged sequences), data-parallel over 8 NeuronCores.

Contract: kernel(**inputs) takes FULL inputs (x [64,2048,300] f32, W [300,300],
b [300]) and returns the FULL output [64, 600] f32 (concat(std, mean) per batch).

Design (v3):
  - Host drops all-padding 128-token tiles (ragged lengths -> ~0.65x work), zeroes
    the remaining pad rows, and packs tiles into batch-aligned GROUPS of 1..4 tiles.
    Every core receives the same sorted multiset of group sizes (dummy all-zero
    groups pad each size class to a multiple of 8), so one SPMD program serves all
    cores while per-core data differs.
  - Per group: one dma_start_transpose loads x^T [128, 3, 128*sz] bf16 straight
    from DRAM (HW xbar transpose; d on partitions).  PE runs 9 matmuls (W-slice
    stationary [128,128] bf16, x^T moving) accumulating pre-activations per
    128-wide output slice in PSUM.  ScalarE applies exact GELU with the bias as a
    per-partition vector and emits the per-slice token-sum via accum_out (fp32
    datapath).  VectorE squares y (scalar_tensor_tensor) and emits sum(y^2) via
    accum_out.  Sums land in per-(slice, group) slots; one DMA returns them.
  - Host epilogue: sums slots per batch, subtracts the analytic contribution of
    the zeroed pad rows (gelu(b) per token; bf16-rounded for the square sum), and
    computes mean/std (unbiased, n<=1 and NaN edge cases per the reference).
"""
import numpy as np
import ml_dtypes

B, T, D = 64, 2048, 300
NCORES = 8
P = 128
DP = 384          # 3 k-slices of 128 (cols 300..383 zero)
KT = 3
MAXSZ = 4         # max tiles per group

bf16 = ml_dtypes.bfloat16

_cache = {}


def _build_nc(group_sizes):
    """One SPMD program for the given per-core group-size list (sorted desc)."""
    from contextlib import ExitStack
    import concourse.tile as tile
    from concourse import mybir, bacc

    f32 = mybir.dt.float32
    bf = mybir.dt.bfloat16
    AF = mybir.ActivationFunctionType
    OP = mybir.AluOpType

    G = len(group_sizes)
    total_tok = 128 * sum(group_sizes)
    PF = 3  # DMA prefetch depth (groups)

    nc = bacc.Bacc("TRN2", target_bir_lowering=False, debug=False)
    x_dram = nc.dram_tensor("x", [total_tok, DP], bf, kind="ExternalInput")
    wt_dram = nc.dram_tensor("wt", [P, KT * KT, P], bf, kind="ExternalInput")
    bn_dram = nc.dram_tensor("bn", [P, KT, G, 6], f32, kind="ExternalOutput")

    offs = np.concatenate([[0], np.cumsum(np.asarray(group_sizes) * 128)])

    with ExitStack() as ctx:
        tc = ctx.enter_context(tile.TileContext(nc))
        const = ctx.enter_context(tc.tile_pool(name="const", bufs=1))
        xtp = ctx.enter_context(tc.tile_pool(name="xtp", bufs=PF + 2))
        yp = ctx.enter_context(tc.tile_pool(name="yp", bufs=6))
        ps_y = ctx.enter_context(tc.tile_pool(name="ps_y", bufs=6, space="PSUM"))
        ps_w = ctx.enter_context(tc.tile_pool(name="ps_w", bufs=1, space="PSUM"))

        wt_sb = const.tile([P, KT * KT, P], bf)
        nc.sync.dma_start(wt_sb[:], wt_dram.ap())
        bnacc = const.tile([P, KT, G, 6], f32)

        # PE warmup: ramp HAM toward 8/8 while the first x DMAs are in flight.
        pwu = ps_w.tile([P, 2 * P], f32, name="pwu", tag="warm")
        for w in range(10):
            nc.tensor.matmul(pwu[:, (w % 2) * P:(w % 2) * P + P],
                             wt_sb[:, w % (KT * KT), :], wt_sb[:, 0, :])

        xts = {}

        def fetch(g):
            sz = group_sizes[g]
            xt = xtp.tile([P, KT, 128 * sz], bf, name=f"xt{g}", tag="xt")
            nc.sync.dma_start_transpose(
                xt[:], x_dram.ap()[offs[g]:offs[g] + 128 * sz, :])
            xts[g] = xt

        for g in range(min(PF, G)):
            fetch(g)

        for g in range(G):
            sz = group_sizes[g]
            n = 128 * sz
            if g + PF < G:
                fetch(g + PF)
            xt = xts.pop(g)
            for i in range(KT):
                py = ps_y.tile([P, n], f32, name=f"py{g}_{i}", tag="py")
                for j in range(KT):
                    nc.tensor.matmul(
                        py[:], wt_sb[:, KT * i + j, :], xt[:, j, :],
                        start=(j == 0), stop=(j == KT - 1),
                    )
                y_sb = yp.tile([P, n], bf, name=f"y{g}_{i}", tag="y")
                nc.scalar.activation(y_sb[:], py[:], AF.Gelu)
                nc.vector.bn_stats(bnacc[:, i, g, :], y_sb[:])
        nc.sync.dma_start(bn_dram.ap()[:], bnacc[:])

    nc.compile()
    return nc


def _plan(valid):
    """Build the packing schedule from the validity mask [B, T].

    Returns (group_sizes, per-core schedules).  Each schedule entry is
    (batch, tile_indices) for one group; batch < 0 marks a dummy group."""
    TPB = T // 128
    vt = valid.reshape(B, TPB, 128)
    keep = vt.any(axis=2)  # [B, TPB] tiles with >=1 valid token

    groups = []  # (size, batch, tile_idx_list)
    for b in range(B):
        tiles = np.nonzero(keep[b])[0].tolist()
        for k in range(0, len(tiles), MAXSZ):
            chunk = tiles[k:k + MAXSZ]
            groups.append((len(chunk), b, chunk))

    # pad each size class to a multiple of NCORES with dummy groups
    from collections import Counter
    cnt = Counter(g[0] for g in groups)
    for s in list(cnt):
        for _ in range((-cnt[s]) % NCORES):
            groups.append((s, -1, []))
    groups.sort(key=lambda g: -g[0])

    # deal round-robin: core c takes groups c, c+8, ... -> identical size lists
    scheds = [[] for _ in range(NCORES)]
    for idx, g in enumerate(groups):
        scheds[idx % NCORES].append(g)
    group_sizes = tuple(g[0] for g in scheds[0])
    for c in range(1, NCORES):
        assert tuple(g[0] for g in scheds[c]) == group_sizes
    return group_sizes, scheds


def _pack_inputs(x, W, b, valid, group_sizes, scheds):
    """Build per-core input maps and the slot->batch bookkeeping."""
    total_tok = 128 * sum(group_sizes)
    vt = valid.reshape(B, T // 128, 128)

    wt = np.zeros((P, KT * KT, P), np.float32)
    Wp = np.zeros((DP, DP), np.float32)
    Wp[:D, :D] = np.asarray(W, np.float32)
    Wp[:D, DP - 1] = np.asarray(b, np.float32)  # bias via ones-column
    for i in range(KT):
        for j in range(KT):
            wt[:, KT * i + j, :] = Wp[128 * i:128 * i + 128,
                                      128 * j:128 * j + 128].T
    wtb = wt.astype(bf16)

    x32 = np.asarray(x, np.float32)
    in_maps = []
    slot_info = []  # per core: list of batch per group slot (-1 = dummy)
    for c in range(NCORES):
        xc = np.zeros((total_tok, DP), bf16)
        info = []
        row = 0
        for (sz, bidx, tiles) in scheds[c]:
            for t in tiles:
                seg = x32[bidx, t * 128:(t + 1) * 128, :]  # [128, 300]
                v = vt[bidx, t]  # [128] bool
                xc[row:row + 128, :D] = np.where(
                    v[:, None], seg, 0.0).astype(bf16)
                xc[row:row + 128, DP - 1] = v.astype(bf16)  # ones col
                row += 128
            row += 128 * (sz - len(tiles))  # dummy groups stay zero
            info.append(bidx)
        in_maps.append({"x": xc, "wt": wtb})
        slot_info.append(info)
    return in_maps, slot_info


def _host_epilogue(res, slot_info, n_valid, b):
    # pad rows are all-zero incl. the ones-column -> y = gelu(0) = 0 exactly,
    # so slot sums need no correction.
    S = np.zeros((B, DP), np.float64)
    Q = np.zeros((B, DP), np.float64)
    for c in range(NCORES):
        bn = np.asarray(res[c]["bn"], np.float64)  # [128, 3, G, 6]
        s_all = bn[..., 0] * bn[..., 1] + bn[..., 3] * bn[..., 4]
        q_all = (bn[..., 2] + bn[..., 0] * np.square(bn[..., 1])
                 + bn[..., 5] + bn[..., 3] * np.square(bn[..., 4]))
        for g, bidx in enumerate(slot_info[c]):
            if bidx < 0:
                continue
            S[bidx] += s_all[:, :, g].T.reshape(DP)
            Q[bidx] += q_all[:, :, g].T.reshape(DP)

    S = S[:, :D]
    Q = Q[:, :D]
    n = n_valid.astype(np.float64)[:, None]

    with np.errstate(divide="ignore", invalid="ignore"):
        mean = S / n
        var = (Q - S * S / n) / np.maximum(n - 1.0, 1.0)
        std = np.where(n > 1.0, np.sqrt(np.maximum(var, 0.0)), 0.0)
    out = np.concatenate([std, mean], axis=-1).astype(np.float32)
    return np.where(np.isnan(out), np.float32(0.0), out)


def _prep(x, W, b):
    x32 = np.asarray(x, np.float32)
    valid = ~np.all(x32 == -1.0, axis=2)  # [B, T]
    group_sizes, scheds = _plan(valid)
    in_maps, slot_info = _pack_inputs(x, W, b, valid, group_sizes, scheds)
    return group_sizes, in_maps, slot_info, valid.sum(1)


def _trace_in_maps(ins):
    group_sizes, in_maps, _, _ = _prep(**ins)
    return in_maps


def kernel(x, W, b):
    from concourse.bass_utils import run_bass_kernel_spmd

    group_sizes, in_maps, slot_info, n_valid = _prep(x, W, b)
    if group_sizes not in _cache:
        _cache[group_sizes] = _build_nc(list(group_sizes))
        _cache["nc"] = _cache[group_sizes]  # latest, for test.py tracing
    nc = _cache[group_sizes]
    _cache["nc"] = nc

    res = run_bass_kernel_spmd(nc, in_maps, core_ids=list(range(NCORES)))
    return _host_epilogue(res.results, slot_info, n_valid, b)
